# revision 60
# baseline (speedup 1.0000x reference)
"""GroupedQueryAttention Bass kernel for 8 TRN2 NeuronCores.

Sharding: core c handles batch b = c//4 and query-row slice s = c%4
(1024 of 4096 rows). Each core computes full K/V for its batch
(redundant across the 4 cores of a batch group) and the final output
rows for its (b, s) slice — no collectives needed.

Per-core math (all matmuls bf16, fp32 PSUM accumulation):
  K^T, V, Q^T projections directly in transposed layout
  RoPE applied on [head_dim, n] layout via partition-block swaps
  S^T = K_g^T-chunks.T @ Q_h^T   (contraction over head_dim=64)
  P^T = exp(S^T / 8)             (no max subtraction; scores are small)
  O^T[128] = [V_g | ones64]^T @ P^T  (64 rows = denominators replicated)
  Ot = O^T[O rows] * recip_approx(O^T[denom rows])  (no broadcast DMA)
  Y = Ot.T @ Wo                  (row-parallel over full Wo)

The attention kc-loop is split into two passes of 16 key-chunks so
pass 0 only depends on the first half of K/V — it overlaps the rest
of the projections. Pass-0 partial O accumulators round-trip DRAM.
"""

from collections import deque

import numpy as np
from ml_dtypes import bfloat16

import concourse.bass as bass
import concourse.mybir as mybir
from concourse import bacc, tile, bass_utils

F32 = mybir.dt.float32
BF16 = mybir.dt.bfloat16
EXP = mybir.ActivationFunctionType.Exp

B, N, D = 2, 4096, 1024
HQ, HKV, HD = 16, 4, 64
G = HQ // HKV          # 4 q heads per kv head
NQ = N // 4            # 1024 q rows per core
N_CORES = 8

_CACHE = {}


def _build():
    nc = bacc.Bacc("TRN2", target_bir_lowering=False, debug=False,
                   num_devices=N_CORES)

    xT_d = nc.dram_tensor("xT", [D, N], BF16, kind="ExternalInput").ap()
    xqT_d = nc.dram_tensor("xqT", [D, NQ], BF16, kind="ExternalInput").ap()
    Wq_d = nc.dram_tensor("Wq", [D, D], BF16, kind="ExternalInput").ap()
    Wk_d = nc.dram_tensor("Wk", [D, 256], BF16, kind="ExternalInput").ap()
    Wv_d = nc.dram_tensor("Wv", [D, 256], BF16, kind="ExternalInput").ap()
    Wo_d = nc.dram_tensor("Wo", [D, D], BF16, kind="ExternalInput").ap()
    cosK_d = nc.dram_tensor("cosK", [128, N], BF16, kind="ExternalInput").ap()
    nsinK_d = nc.dram_tensor("nsinK", [128, N], BF16, kind="ExternalInput").ap()
    cosQ_d = nc.dram_tensor("cosQ", [128, NQ], BF16, kind="ExternalInput").ap()
    nsinQ_d = nc.dram_tensor("nsinQ", [128, NQ], BF16, kind="ExternalInput").ap()
    out_d = nc.dram_tensor("out", [NQ, D], F32, kind="ExternalOutput").ap()

    NKC = N // 128          # 32 key chunks
    VBLK = 192              # V pair block: [V_even(64) | ones(64) | V_odd(64)]

    with tile.TileContext(nc) as tc:
      with tc.tile_pool(name="persist", bufs=1) as pp, \
           tc.tile_pool(name="dram", bufs=1, space="DRAM") as dp, \
           tc.tile_pool(name="pb", bufs=1) as pb, \
           tc.tile_pool(name="pbp", bufs=1, space="PSUM") as pbp:
        with tc.tile_pool(name="pa", bufs=1) as pa, \
             tc.tile_pool(name="pap", bufs=1, space="PSUM") as pap:

            # ---- persistent SBUF tensors ----
            Qrt = [pp.tile([128, 4 * NQ], BF16, tag=f"qrt{t}", name=f"qrt{t}")
                   for t in range(2)]
            Krt = [pp.tile([128, N], BF16, tag=f"krt{t}", name=f"krt{t}")
                   for t in range(2)]
            Vsb = pp.tile([128, NKC * 2 * VBLK], BF16, tag="vsb", name="vsb")
            Ot = [pp.tile([128, NQ], BF16, tag=f"ot{i}", name=f"ot{i}")
                  for i in range(8)]
            Wo16s = pp.tile([128, 8 * D], BF16, tag="wo", name="wo")

            # ---- PE warm-up: dummy matmuls while first DMAs land ----
            warm = pa.tile([128, 512], BF16, tag="warm", name="warm")
            nc.vector.memset(warm[:], 0.0)
            for w in range(8):
                wp = pap.tile([128, 512], F32, tag="mm", bufs=2, name="wp")
                nc.tensor.matmul(wp[:], warm[:, 0:128], warm[:],
                                 start=True, stop=True)

            # ones block (cols 64:128) in every V pair block: the AV matmul
            # then emits softmax denominators replicated across 64 partitions
            vview = Vsb.rearrange("p (b c) -> p b c", c=VBLK)[:, :, 64:128]
            nc.vector.memset(vview, 1.0)

            # input tiles (DMAs are emitted later in priority order).
            # 8-slice loads go as ONE 3D-AP DMA each: the DMA queue is
            # descriptor-rate bound at the head, not bandwidth bound.
            xqs = pa.tile([128, 8 * NQ], BF16, tag="xqs", name="xqs")
            cosQ = pa.tile([128, NQ], BF16, tag="cosq", name="cosq")
            nsinQ = pa.tile([128, NQ], BF16, tag="nsinq", name="nsinq")
            Wks = pa.tile([128, 8 * 256], BF16, tag="wks", name="wks")
            Wvs = pa.tile([128, 8 * 256], BF16, tag="wvs", name="wvs")

            def dram8(src, ncols, c0, w):
                """[8*128, ncols] DRAM tensor -> [128, 8, w] AP at col c0."""
                return bass.AP(src.tensor, src.offset + c0,
                               [[ncols, 128], [128 * ncols, 8], [1, w]])

            def xq(dc):
                return xqs[:, dc * NQ:(dc + 1) * NQ]

            def rope(psum, cos_s, nsin_s, w):
                """Return (t1, tmp): rope result = t1 + tmp (caller adds).

                One 1x PSUM read, then everything in bf16 SBUF (4x DVE)."""
                kb = pa.tile([128, w], BF16, tag="kb", bufs=3, name="kb")
                nc.vector.tensor_copy(kb[:], psum[:])
                tmp = pa.tile([128, w], BF16, tag="rtmp", bufs=3, name="rtmp")
                for blk in range(4):
                    src = (blk ^ 1) * 32
                    nc.vector.tensor_copy(tmp[blk * 32:(blk + 1) * 32, :],
                                          kb[src:src + 32, :])
                t1 = pa.tile([128, w], BF16, tag="rt1", bufs=3, name="rt1")
                nc.vector.tensor_mul(t1[:], kb[:], cos_s)
                nc.vector.tensor_mul(tmp[:], tmp[:], nsin_s)
                return t1, tmp

            # ---- projection slices (each a small closure: one insert) ----
            wq_dma, xtd, ckd = {}, {}, {}

            def qdma(hp):
                def f():
                    wq = pa.tile([128, 8 * 128], BF16, tag="wq", bufs=2,
                                 name="wq")
                    wq_dma[hp] = wq
                    nc.sync.dma_start(
                        wq.rearrange("p (a c) -> p a c", c=128),
                        dram8(Wq_d, D, hp * 128, 128))
                return f

            pqd = {}

            def qmmh(hp, nch, half):   # half a Q projection (4 dc chunks)
                def f():
                    if half == 0:
                        pqd[(hp, nch)] = pap.tile([128, 512], F32, tag="mm",
                                                  bufs=2, name="pq")
                    pq = pqd[(hp, nch)]
                    wq = wq_dma[hp]
                    for dc in range(4 * half, 4 * half + 4):
                        nc.tensor.matmul(
                            pq[:], wq[:, dc * 128:(dc + 1) * 128],
                            xq(dc)[:, nch * 512:(nch + 1) * 512],
                            start=(dc == 0), stop=(dc == 7))
                    if half == 0:
                        return
                    g = hp // 2
                    t = g // 2
                    base = (g % 2) * 64
                    hi = 2 * hp - 4 * g
                    c0 = nch * 512
                    t1, tmp = rope(pq, cosQ[:, c0:c0 + 512],
                                   nsinQ[:, c0:c0 + 512], 512)
                    d0 = hi * NQ + c0
                    d1 = (hi + 1) * NQ + c0
                    nc.vector.tensor_add(
                        Qrt[t][base:base + 64, d0:d0 + 512],
                        t1[0:64, :], tmp[0:64, :])
                    nc.vector.tensor_add(
                        Qrt[t][base:base + 64, d1:d1 + 512],
                        t1[64:128, :], tmp[64:128, :])
                return f

            def qmm(hp, nch):
                h0, h1 = qmmh(hp, nch, 0), qmmh(hp, nch, 1)

                def f():
                    h0()
                    h1()
                return f

            def kdma(nch):             # xT + rope-table DMAs for one chunk
                def f():
                    c0 = nch * 512
                    xts = pa.tile([128, 8 * 512], BF16, tag="xts", bufs=2,
                                  name="xts")
                    nc.sync.dma_start(
                        xts.rearrange("p (a c) -> p a c", c=512),
                        dram8(xT_d, N, c0, 512))
                    ck = pa.tile([128, 512], BF16, tag="ck", bufs=2,
                                 name="ck")
                    nk = pa.tile([128, 512], BF16, tag="nk", bufs=2,
                                 name="nk")
                    nc.sync.dma_start(ck[:], cosK_d[:, c0:c0 + 512])
                    nc.sync.dma_start(nk[:], nsinK_d[:, c0:c0 + 512])
                    xtd[nch] = xts
                    ckd[nch] = (ck, nk)
                return f

            pkd = {}

            def kmmh(nch, pt, half):   # half a K projection (4 dc chunks)
                def f():
                    if half == 0:
                        pkd[(nch, pt)] = pap.tile([128, 512], F32, tag="mm",
                                                  bufs=2, name="pk")
                    pk = pkd[(nch, pt)]
                    xts = xtd[nch]
                    for dc in range(4 * half, 4 * half + 4):
                        nc.tensor.matmul(
                            pk[:],
                            Wks[:, dc * 256 + pt * 128:
                                dc * 256 + (pt + 1) * 128],
                            xts[:, dc * 512:(dc + 1) * 512],
                            start=(dc == 0), stop=(dc == 7))
                    if half == 0:
                        return
                    c0 = nch * 512
                    ck, nk = ckd[nch]
                    t1, tmp = rope(pk, ck[:], nk[:], 512)
                    nc.vector.tensor_add(Krt[pt][:, c0:c0 + 512],
                                         t1[:], tmp[:])
                return f

            def kmm(nch, pt):
                h0, h1 = kmmh(nch, pt, 0), kmmh(nch, pt, 1)

                def f():
                    h0()
                    h1()
                return f

            pvd = {}

            def vvch(nch, vc, half):   # half a V projection (4 dc chunks)
                def f():
                    if half == 0:
                        pvd[(nch, vc)] = pap.tile([128, 256], F32, tag="mm",
                                                  bufs=2, name="pv")
                    pv = pvd[(nch, vc)]
                    xts = xtd[nch]
                    for dc in range(4 * half, 4 * half + 4):
                        nc.tensor.matmul(
                            pv[:],
                            xts[:, dc * 512 + vc * 128:
                                dc * 512 + (vc + 1) * 128],
                            Wvs[:, dc * 256:(dc + 1) * 256],
                            start=(dc == 0), stop=(dc == 7))
                    if half == 0:
                        return
                    kc = nch * 4 + vc
                    for pr in range(2):
                        off = (kc * 2 + pr) * VBLK
                        dst = bass.AP(Vsb.tensor, Vsb.offset + off,
                                      [Vsb.ap[0], [128, 2], [1, 64]])
                        src = pv[:, pr * 128:(pr + 1) * 128].rearrange(
                            "p (g c) -> p g c", c=64)
                        nc.vector.tensor_copy(dst, src)
                return f

            def vvc(nch, vc):
                h0, h1 = vvch(nch, vc, 0), vvch(nch, vc, 1)

                def f():
                    h0()
                    h1()
                return f

            # ---- attention iterations, two-phase for seam overlap ----
            # emit_head: first two score chunks + first exp, emitted BEFORE
            #   the previous iteration's tail so the scalar engine crosses
            #   iteration seams without a bubble.
            # emit_body: rest of the kc stream; the insert hook drops queued
            #   out-proj work into the PE slack every `stride` chunks.
            # emit_tail: last AV pair + flush or normalize.
            opart = {}
            norm_done = {}

            def make_it(pt, hi, qh, kc_lo, kc_hi, mode):
                g0, g1 = 2 * pt, 2 * pt + 1
                q0 = hi * NQ + qh * 512
                kcl = list(range(kc_lo, kc_hi))
                S, P = {}, {}

                def scores(kc):
                    st = pbp.tile([128, 1024], F32, tag="st", bufs=2,
                                  name="st")
                    nc.tensor.matmul(
                        st[:, 0:512],
                        Krt[pt][0:64, kc * 128:(kc + 1) * 128],
                        Qrt[pt][0:64, q0:q0 + 512],
                        start=True, stop=True)
                    nc.tensor.matmul(
                        st[:, 512:1024],
                        Krt[pt][64:128, kc * 128:(kc + 1) * 128],
                        Qrt[pt][64:128, q0:q0 + 512],
                        start=True, stop=True)
                    S[kc] = st

                def expo(kc):
                    pT = pb.tile([128, 1024], BF16, tag="pT", bufs=4,
                                 name="pT")
                    nc.scalar.activation(pT[:], S.pop(kc)[:], EXP,
                                         scale=0.125)
                    P[kc] = pT

                def av(kc, start, stop):
                    off = (kc * 2 + pt) * VBLK
                    pT = P.pop(kc)
                    nc.tensor.matmul(
                        P["oA"][:], Vsb[:, off:off + 128], pT[:, 0:512],
                        start=start, stop=stop)
                    nc.tensor.matmul(
                        P["oB"][:], Vsb[:, off + 64:off + 192],
                        pT[:, 512:1024], start=start, stop=stop)

                def emit_head():
                    scores(kcl[0])
                    if len(kcl) > 1:
                        scores(kcl[1])
                    expo(kcl[0])
                    if len(kcl) > 1:
                        expo(kcl[1])

                def emit_body(inserts=None, stride=4):
                    P["oA"] = pbp.tile([128, 512], F32, tag="oA", bufs=1,
                                       name="oA")
                    P["oB"] = pbp.tile([128, 512], F32, tag="oB", bufs=1,
                                       name="oB")
                    if mode == "combine":
                        for i, key in enumerate(("oiA", "oiB")):
                            oi = pb.tile([128, 512], BF16, tag=key, bufs=2,
                                         name=key)
                            nc.sync.dma_start(
                                oi[:], opart[(pt, hi, qh, i)][:])
                            P[key] = oi
                    n = len(kcl)
                    for j in range(n):
                        if (inserts is not None and len(inserts) and j >= 2
                                and j % stride == stride - 1):
                            inserts.popleft()[1]()
                        if j + 2 < n:
                            scores(kcl[j + 2])
                        if 2 <= j + 1 < n:
                            expo(kcl[j + 1])
                        if j < n - 1:
                            av(kcl[j], start=(j == 0), stop=False)

                def emit_tail():
                    n = len(kcl)
                    av(kcl[-1], start=(n == 1), stop=True)
                    oA, oB = P["oA"], P["oB"]
                    if mode == "flush":
                        for i, o in enumerate((oA, oB)):
                            osb = pb.tile([128, 512], BF16, tag="osb",
                                          bufs=2, name="osb")
                            nc.vector.tensor_copy(osb[:], o[:])
                            od = dp.tile([128, 512], BF16,
                                         tag=f"op{pt}{hi}{qh}{i}",
                                         name=f"op{pt}{hi}{qh}{i}")
                            opart[(pt, hi, qh, i)] = od
                            nc.sync.dma_start(od[:], osb[:])
                        return
                    # normalize: side A has O rows 0:64 / denom 64:128;
                    # side B is flipped. In combine mode drain the PSUM
                    # accumulator with ONE copy first so the next
                    # iteration's AV can reuse the bank ~0.7us sooner.
                    for o, okey, g, dlo in ((oA, "oiA", g0, 64),
                                            (oB, "oiB", g1, 0)):
                        olo = 64 - dlo
                        head = 4 * g + hi
                        hc, row = head // 2, (head % 2) * 64
                        ts_ = pb.tile([64, 512], F32, tag="ts", bufs=2,
                                      name="ts")
                        dd = pb.tile([64, 512], F32, tag="dd", bufs=2,
                                     name="dd")
                        if mode == "combine":
                            ob = pb.tile([128, 512], BF16, tag="osb",
                                         bufs=2, name="osb")
                            nc.vector.tensor_copy(ob[:], o[:])
                            oi = P[okey]
                            nc.vector.tensor_add(ts_[:], ob[olo:olo + 64, :],
                                                 oi[olo:olo + 64, :])
                            nc.vector.tensor_add(dd[:], ob[dlo:dlo + 64, :],
                                                 oi[dlo:dlo + 64, :])
                        else:
                            nc.vector.tensor_copy(ts_[:],
                                                  o[olo:olo + 64, :])
                            nc.vector.tensor_copy(dd[:],
                                                  o[dlo:dlo + 64, :])
                        rb = pb.tile([64, 512], F32, tag="rb", bufs=2,
                                     name="rb")
                        nc.vector.reciprocal_approx_fast(rb[:], dd[:])
                        norm_done[(qh, head)] = nc.vector.tensor_mul(
                            Ot[hc][row:row + 64, qh * 512:(qh + 1) * 512],
                            ts_[:], rb[:])

                return emit_head, emit_body, emit_tail

            pend_tail = [None]

            def run_it(pt, hi, qh, kc_lo, kc_hi, mode, inserts=None,
                       stride=4):
                h, b, t = make_it(pt, hi, qh, kc_lo, kc_hi, mode)
                h()
                if pend_tail[0] is not None:
                    pend_tail[0]()
                b(inserts, stride)
                pend_tail[0] = t

            # pass-0 depth per (pt, hi): grows as projections land, so the
            # scalar engine (exp) never starves while the PE does projections
            B = {(0, 0): 4, (0, 1): 12, (0, 2): 12, (0, 3): 16,
                 (1, 0): 20, (1, 1): 24, (1, 2): 28, (1, 3): 32}

            def wo_dma():
                nc.sync.dma_start(
                    Wo16s.rearrange("p (a c) -> p a c", c=D),
                    dram8(Wo_d, D, 0, D))

            # ---- warm-path emission (PE FIFO order) ----
            # DMAs first, in arrival-priority order (first-exp chain is
            # Wk -> xt0 -> rope tables -> wq -> xq -> cos/nsin; Wv later)
            nc.sync.dma_start(Wks.rearrange("p (a c) -> p a c", c=256),
                              dram8(Wk_d, 256, 0, 256))
            kdma(0)()
            qdma(0)()
            qdma(2)()
            nc.sync.dma_start(xqs.rearrange("p (a c) -> p a c", c=NQ),
                              dram8(xqT_d, NQ, 0, NQ))
            nc.sync.dma_start(cosQ[:], cosQ_d[:])
            nc.sync.dma_start(nsinQ[:], nsinQ_d[:])
            nc.sync.dma_start(Wvs.rearrange("p (a c) -> p a c", c=256),
                              dram8(Wv_d, 256, 0, 256))
            # minimal PE prefix for the first iteration (K-chain first:
            # its DMAs land earliest)
            kmm(0, 0)()
            kmm(0, 1)()
            qmm(0, 0)()
            qmm(2, 0)()
            vvc(0, 0)()
            vvc(0, 1)()
            vvc(0, 2)()
            vvc(0, 3)()
            run_it(0, 0, 0, 0, B[(0, 0)], "flush")
            qmm(0, 1)()
            qmm(2, 1)()
            run_it(0, 0, 1, 0, B[(0, 0)], "flush")
            kdma(1)()
            kmm(1, 0)()
            kmm(1, 1)()
            for vc in range(4):
                vvc(1, vc)()
            kdma(2)()
            # remaining projections ride the insert queue
            w0q = deque()

            def grp(tag, *fns):
                for f in fns:
                    w0q.append((tag, f))

            def drain(tag):
                while any(t == tag for t, _ in w0q):
                    w0q.popleft()[1]()

            def qg(tag, hp):
                grp(tag, qdma(hp))
                for n_ in (0, 1):
                    grp(tag, qmmh(hp, n_, 0), qmmh(hp, n_, 1))

            def kg(tag, nch):
                for pt_ in (0, 1):
                    grp(tag, kmmh(nch, pt_, 0), kmmh(nch, pt_, 1))
                for vc_ in range(4):
                    grp(tag, vvch(nch, vc_, 0), vvch(nch, vc_, 1))

            # kv2 pops just-in-time inside (0,1,0): K-pt0 first (scores
            # need it by j=6), V halves next (AV kc8..10 at j=8..10),
            # K-pt1 last (not needed until the pt1 iterations)
            grp("kv2", kmmh(2, 0, 0), kmmh(2, 0, 1))
            for vc_ in range(4):
                grp("kv2", vvch(2, vc_, 0), vvch(2, vc_, 1))
            grp("kv2", kmmh(2, 1, 0), kmmh(2, 1, 1))
            qg("q13", 1)
            qg("q13", 3)
            grp("kv3", kdma(3))
            qg("q46", 4)
            qg("q46", 6)
            kg("kv3", 3)
            grp("kv4", kdma(4))
            kg("kv4", 4)
            qg("q57", 5)
            qg("q57", 7)
            grp("kv5", kdma(5))
            kg("kv5", 5)
            grp("kv6", kdma(6))
            kg("kv6", 6)
            grp("kv7", kdma(7))
            kg("kv7", 7)
            grp("wo", wo_dma)

            run_it(0, 1, 0, 0, B[(0, 1)], "flush", w0q, 1)
            run_it(0, 1, 1, 0, B[(0, 1)], "flush", w0q, 1)
            drain("q13")
            run_it(0, 2, 0, 0, B[(0, 2)], "flush", w0q, 1)
            run_it(0, 2, 1, 0, B[(0, 2)], "flush", w0q, 1)
            drain("kv3")
            run_it(0, 3, 0, 0, B[(0, 3)], "flush", w0q, 2)
            run_it(0, 3, 1, 0, B[(0, 3)], "flush", w0q, 2)
            drain("q46")
            drain("kv4")
            run_it(1, 0, 0, 0, B[(1, 0)], "flush", w0q, 2)
            run_it(1, 0, 1, 0, B[(1, 0)], "flush", w0q, 2)
            drain("kv5")
            run_it(1, 1, 0, 0, B[(1, 1)], "flush", w0q, 3)
            run_it(1, 1, 1, 0, B[(1, 1)], "flush", w0q, 3)
            drain("q57")
            drain("kv6")
            run_it(1, 2, 0, 0, B[(1, 2)], "flush", w0q, 3)
            run_it(1, 2, 1, 0, B[(1, 2)], "flush", w0q, 3)
            drain("kv7")
            run_it(1, 3, 0, 0, 32, "single", w0q, 3)
            run_it(1, 3, 1, 0, 32, "single", w0q, 3)
            drain("wo")
            assert not w0q
            _run_it = run_it           # keep usable after pa closes

        # pa/pap released: 2 PSUM banks free for the output projection
        with tc.tile_pool(name="pc", bufs=1) as pc, \
             tc.tile_pool(name="pcp", bufs=1, space="PSUM") as pcp:

            ysA = {}

            def op_mm(yp, qc, dh, hc, start, stop, pin=True):
                qh = 0 if qc < 4 else 1
                mm = nc.tensor.matmul(
                    yp[:], Ot[hc][:, qc * 128:(qc + 1) * 128],
                    Wo16s[:, hc * D + dh * 512:hc * D + (dh + 1) * 512],
                    start=start, stop=stop)
                if not pin:
                    return
                for hd in (2 * hc, 2 * hc + 1):
                    dep = norm_done.get((qh, hd))
                    if dep is not None:
                        bass._add_dep_helper(mm.ins, dep.ins, sync=True,
                                             reason="outproj after norm")

            def op_round1(qc, dh, hcs=(4, 5, 6, 7)):   # -> ysA (SBUF f32)
                def emit():
                    yp = pcp.tile([128, 512], F32, tag="y", bufs=2,
                                  name="yp")
                    for i, hc in enumerate(hcs):
                        op_mm(yp, qc, dh, hc, i == 0, i == len(hcs) - 1,
                              pin=(i < 2))
                    ya = pc.tile([128, 512], F32, tag=f"ysA{qc}{dh}",
                                 name=f"ysA{qc}{dh}")
                    ysA[(qc, dh)] = ya
                    nc.vector.tensor_copy(ya[:], yp[:])
                return emit

            def op_round2(qc, dh, hcs=(0, 1, 3, 2)):   # + ysA -> out
                def emit():
                    yp = pcp.tile([128, 512], F32, tag="y", bufs=2,
                                  name="yp")
                    for i, hc in enumerate(hcs):
                        op_mm(yp, qc, dh, hc, i == 0, i == len(hcs) - 1,
                              pin=(i < 3))
                    ys = pc.tile([128, 512], F32, tag="ys", bufs=2,
                                 name="ys")
                    nc.vector.tensor_add(ys[:], yp[:], ysA[(qc, dh)][:])
                    nc.sync.dma_start(
                        out_d[qc * 128:(qc + 1) * 128,
                              dh * 512:(dh + 1) * 512], ys[:])
                return emit

            # pass 1: pt1 iterations first so pt1 heads complete early,
            # then the long pt0 iterations whose slack absorbs out-proj
            # work. qh1 runs (0,2)/(0,3) before (0,0)/(0,1) so only
            # hc0/hc2 complete last: its round-1 covers 6 of 8 hc and the
            # end-of-kernel round shrinks to 2 matmuls per group.
            seq0 = [(1, 0), (1, 1), (1, 2), (0, 0), (0, 1), (0, 2), (0, 3)]
            seq1 = [(1, 0), (1, 1), (1, 2), (0, 2), (0, 3), (0, 0), (0, 1)]
            ins_q = deque()
            for idx, (pt, hi) in enumerate(seq0):       # qh = 0
                if idx == 3:
                    for qc in range(0, 4):
                        for dh in range(2):
                            ins_q.append(("op", op_round1(qc, dh)))
                _run_it(pt, hi, 0, B[(pt, hi)], 32, "combine", ins_q, 5)
            for idx, (pt, hi) in enumerate(seq1):       # qh = 1
                if idx == 0:
                    for qc in range(0, 4):              # finish qh0 groups
                        for dh in range(2):
                            ins_q.append(("op", op_round2(qc, dh)))
                if idx == 5:
                    for qc in range(4, 8):
                        for dh in range(2):
                            ins_q.append(
                                ("op",
                                 op_round1(qc, dh, (1, 3, 4, 5, 6, 7))))
                _run_it(pt, hi, 1, B[(pt, hi)], 32, "combine", ins_q, 5)
            pend_tail[0]()             # last attention tail + normalize
            pend_tail[0] = None
            while ins_q:
                ins_q.popleft()[1]()
            for qc in range(4, 8):
                for dh in range(2):
                    op_round2(qc, dh, (0, 2))()

    nc.compile()
    return nc


def get_nc():
    if "nc" not in _CACHE:
        _CACHE["nc"] = _build()
    return _CACHE["nc"]


def _rope_tables():
    inv_freq = 1.0 / (10000.0 ** (np.arange(0, HD, 2, dtype=np.float32) / HD))
    t = np.arange(N, dtype=np.float32)
    freqs = np.outer(t, inv_freq)
    emb = np.concatenate([freqs, freqs], -1)        # [N, HD]
    return np.cos(emb).astype(np.float32), np.sin(emb).astype(np.float32)


def make_in_maps(x, Wq, Wk, Wv, Wo):
    cos, sin = _rope_tables()
    cosT = np.ascontiguousarray(cos.T)              # [64, N]
    nsinT = np.ascontiguousarray(sin.T)
    nsinT[0:32] = -nsinT[0:32]
    cosK = np.vstack([cosT, cosT])                  # [128, N]
    nsinK = np.vstack([nsinT, nsinT])

    bf = lambda a: np.ascontiguousarray(a).astype(bfloat16)
    Wq16, Wk16, Wv16, Wo16 = bf(Wq), bf(Wk), bf(Wv), bf(Wo)
    cosK16, nsinK16 = bf(cosK), bf(nsinK)

    in_maps = []
    for c in range(N_CORES):
        b, s = c // 4, c % 4
        xT = bf(x[b].T)                             # [D, N]
        xqT = bf(x[b, s * NQ:(s + 1) * NQ, :].T)    # [D, NQ]
        in_maps.append({
            "xT": xT, "xqT": xqT,
            "Wq": Wq16, "Wk": Wk16, "Wv": Wv16, "Wo": Wo16,
            "cosK": cosK16, "nsinK": nsinK16,
            "cosQ": bf(cosK[:, s * NQ:(s + 1) * NQ]),
            "nsinQ": bf(nsinK[:, s * NQ:(s + 1) * NQ]),
        })
    return in_maps


def assemble(results):
    out = np.zeros((B, N, D), np.float32)
    for c in range(N_CORES):
        b, s = c // 4, c % 4
        out[b, s * NQ:(s + 1) * NQ, :] = results[c]["out"]
    return out


def kernel(x, Wq, Wk, Wv, Wo):
    nc = get_nc()
    in_maps = make_in_maps(np.asarray(x, np.float32), np.asarray(Wq, np.float32),
                           np.asarray(Wk, np.float32), np.asarray(Wv, np.float32),
                           np.asarray(Wo, np.float32))
    res = bass_utils.run_bass_kernel_spmd(nc, in_maps,
                                          core_ids=list(range(N_CORES)))
    return assemble(res.results)

